# revision 6
# baseline (speedup 1.0000x reference)
"""Trainium2 Bass kernel for nn_CrossAttention_44693429682227.

Math (reference):
    q = (x @ Wq.T) / E**0.25, reshaped (b, t, H, E)
    scores = q @ keys.T over a shared bank of N=50000 (key, scalar-value) pairs
    attn = softmax(scores, axis=-1)
    out = mean_h(attn @ values) + curiosity  -> (b, t, 1)

Because values are scalars, out_row = (sum_n exp(s_n) * v_n) / (sum_n exp(s_n)).
Scores are bounded (|s| <~ 20), so f32 exp never overflows and no max-
subtraction is needed; numerator and denominator partials are exact to merge
across key-bank shards.

Distribution: the key bank is sharded 8 ways (6528 keys/core incl. padding);
every core computes the projection for all 4096 (b,t,h) query rows (replicated,
cheap) and full partial num/den sums over its shard. Host merges partials.

Per-core program (all static/unrolled, Tile-scheduled):
  - PE: qT projection (8 heads x 8 k-chunks), then per (head, key-block):
        scoresT[block] = keysT_block.T @ qT_head   (out: [128 keys, 512 qrows])
        ndacc += vb_block.T @ exp(scoresT)         (vb = [v_hi, v_lo, mask])
  - ACT: exp over groups of GB=3 PSUM banks per instruction (amortizes the
    ~352-cycle ACTIVATE overhead)
  - output: [3, H, 512] num_hi/num_lo/den partials -> DRAM, merged on host

kernel.py is self-contained: shapes/sharding hardcoded, no sibling imports.
"""

import os
import sys
from contextlib import ExitStack

import numpy as np

if "/opt/trn_rl_repo" not in sys.path:
    sys.path.insert(0, "/opt/trn_rl_repo")

import ml_dtypes

# Problem shapes (hardcoded per contract)
B, T = 4, 128
BT = B * T            # 512 query (b,t) rows
HIN = 1024
H, E = 8, 128
N = 50000
NCORES = 8

# Sharding / tiling
GB = 3                # key-blocks (128 keys each) per ACTIVATE group
NGRP = 17             # groups per core
NBLK = GB * NGRP      # 51 key-blocks per core
KC = NBLK * 128       # 6528 keys per core
NPAD = KC * NCORES    # 52224 padded bank size
KCH = HIN // 128      # 8 contraction chunks for the projection

# dtype knobs: "bf16" | "f32r" | "f32"
MM_DT = os.environ.get("KMM_DT", "bf16")      # scores + num/den matmuls
PROJ_DT = os.environ.get("KPROJ_DT", "f32")   # q projection matmul
TRACE = bool(int(os.environ.get("KTRACE", "0")))

LAST_RESULTS = None   # BassKernelResults of the most recent run (for test.py)

_cache = {}


def _install_ntff_hook():
    """Register the axon NTFF profile hook that this image's antenv lacks.

    bass_utils reads it via ``antenv.axon_hooks.get_axon_ntff_profile_hook``;
    we synthesize that module around trn_agent_boot's ctypes implementation.
    Also soften ``upload_artifacts`` (no bucket access needed for local runs).
    """
    import types

    if "antenv.axon_hooks" in sys.modules:
        return
    try:
        from trn_agent_boot.trn_boot import _ntff_profile_via_ctypes

        hook = _ntff_profile_via_ctypes("/opt/axon/libaxon_pjrt.so")
    except Exception:
        hook = None
    mod = types.ModuleType("antenv.axon_hooks")
    mod.get_axon_ntff_profile_hook = lambda: hook
    sys.modules["antenv.axon_hooks"] = mod

    from concourse import bass_utils as bu

    orig_upload = bu.upload_artifacts

    def safe_upload(tmpdir):
        try:
            return orig_upload(tmpdir)
        except Exception as e:
            return f"upload-skipped ({type(e).__name__})"

    bu.upload_artifacts = safe_upload


def _np_dt(tag):
    return ml_dtypes.bfloat16 if tag == "bf16" else np.float32


def _build(mm, proj):
    import concourse.bass as bass
    import concourse.tile as tile
    from concourse import bacc, mybir

    f32 = mybir.dt.float32
    f32r = mybir.dt.float32r
    bf16 = mybir.dt.bfloat16

    def decl(tag):
        return bf16 if tag == "bf16" else f32

    def cast(ap, tag):
        return ap.bitcast(f32r) if tag == "f32r" else ap

    mm_decl = decl(mm)
    proj_decl = decl(proj)

    nc = bacc.Bacc(trn_type="TRN2", target_bir_lowering=False, debug=False)

    xt_d = nc.dram_tensor("xt", [HIN, BT], proj_decl, kind="ExternalInput")
    wqt_d = nc.dram_tensor("wqt", [HIN, H * E], proj_decl, kind="ExternalInput")
    keyst_d = nc.dram_tensor("keyst", [E, KC], mm_decl, kind="ExternalInput")
    vb_d = nc.dram_tensor("vb", [128, NBLK * 3], mm_decl, kind="ExternalInput")
    nd_d = nc.dram_tensor("nd_out", [3, H * BT], f32, kind="ExternalOutput")

    Exp = mybir.ActivationFunctionType.Exp

    with tile.TileContext(nc) as tc, ExitStack() as ctx:
        singles = ctx.enter_context(tc.tile_pool(name="singles", bufs=1))
        epool = ctx.enter_context(tc.tile_pool(name="epool", bufs=3))
        ps_s = ctx.enter_context(tc.tile_pool(name="ps_s", bufs=2, space="PSUM"))
        ps_sm = ctx.enter_context(tc.tile_pool(name="ps_sm", bufs=2, space="PSUM"))

        # ---- persistent SBUF loads ----
        xt_sb = singles.tile([128, KCH, BT], proj_decl)
        for k in range(KCH):
            nc.sync.dma_start(out=xt_sb[:, k, :], in_=xt_d.ap()[128 * k:128 * (k + 1), :])

        wq_sb = singles.tile([128, KCH, H, E], proj_decl)
        wq_view = wqt_d.ap().rearrange("(k p) o -> p k o", p=128)
        for h in range(H):
            nc.sync.dma_start(out=wq_sb[:, :, h, :], in_=wq_view[:, :, E * h:E * (h + 1)])

        keyst_sb = singles.tile([128, KC], mm_decl)
        kchunk = KC // 8
        for i in range(8):
            nc.sync.dma_start(
                out=keyst_sb[:, kchunk * i:kchunk * (i + 1)],
                in_=keyst_d.ap()[:, kchunk * i:kchunk * (i + 1)],
            )

        vb_sb = singles.tile([128, NBLK, 3], mm_decl)
        nc.sync.dma_start(out=vb_sb, in_=vb_d.ap().rearrange("p (b c) -> p b c", c=3))

        qt_sb = singles.tile([128, H, BT], mm_decl)
        out_sb = singles.tile([3, H, BT], f32)

        # ---- q projection: qT[e, (h, bt)] ----
        for h in range(H):
            q_ps = ps_sm.tile([128, BT], f32, tag="sm", name=f"q_ps{h}")
            for k in range(KCH):
                nc.tensor.matmul(
                    q_ps,
                    lhsT=cast(wq_sb[:, k, h, :], proj),
                    rhs=cast(xt_sb[:, k, :], proj),
                    start=(k == 0),
                    stop=(k == KCH - 1),
                )
            nc.vector.tensor_copy(qt_sb[:, h, :], q_ps)

        # ---- main: scores -> exp -> num/den accumulation ----
        for h in range(H):
            nd_ps = ps_sm.tile([3, BT], f32, tag="sm", name=f"nd_ps{h}")
            for g in range(NGRP):
                s_ps = ps_s.tile([128, GB, BT], f32, tag="s", name=f"s_ps_{h}_{g}")
                for j in range(GB):
                    b = g * GB + j
                    nc.tensor.matmul(
                        s_ps[:, j, :],
                        lhsT=cast(keyst_sb[:, 128 * b:128 * (b + 1)], mm),
                        rhs=cast(qt_sb[:, h, :], mm),
                        start=True,
                        stop=True,
                    )
                eT = epool.tile([128, GB, BT], mm_decl, tag="e", name=f"eT_{h}_{g}")
                nc.scalar.activation(eT, s_ps, Exp)
                for j in range(GB):
                    b = g * GB + j
                    nc.tensor.matmul(
                        nd_ps,
                        lhsT=cast(vb_sb[:, b, :], mm),
                        rhs=cast(eT[:, j, :], mm),
                        start=(b == 0),
                        stop=(b == NBLK - 1),
                    )
            nc.vector.tensor_copy(out_sb[:, h, :], nd_ps)

        nc.sync.dma_start(out=nd_d.ap(), in_=out_sb.rearrange("p h b -> p (h b)"))

    nc.compile()
    return nc


def _prep_inputs(x, Wq, keys, values, mm, proj):
    mm_np = _np_dt(mm)
    proj_np = _np_dt(proj)

    xT = np.ascontiguousarray(
        np.asarray(x, dtype=np.float32).reshape(BT, HIN).T
    ).astype(proj_np)
    # fold the 1/E**0.25 query scale into Wq
    wqT = np.ascontiguousarray(
        (np.asarray(Wq, dtype=np.float32) * np.float32(E ** -0.25)).T
    ).astype(proj_np)

    keys_pad = np.zeros((NPAD, E), dtype=np.float32)
    keys_pad[:N] = np.asarray(keys, dtype=np.float32)
    keysT = np.ascontiguousarray(keys_pad.T).astype(mm_np)  # [E, NPAD]

    v = np.asarray(values, dtype=np.float32)
    v_pad = np.zeros(NPAD, dtype=np.float32)
    v_pad[:N] = v
    mask = np.zeros(NPAD, dtype=np.float32)
    mask[:N] = 1.0

    if mm == "bf16":
        v_hi32 = v_pad.astype(ml_dtypes.bfloat16).astype(np.float32)
        v_lo32 = v_pad - v_hi32
    else:
        v_hi32 = v_pad
        v_lo32 = np.zeros_like(v_pad)

    # vb[core][p, blk, 3] with p = key index within 128-block
    def shard_cols(a):  # [NPAD] -> [NCORES, 128, NBLK]
        return a.reshape(NCORES, NBLK, 128).transpose(0, 2, 1)

    vb = np.stack(
        [shard_cols(v_hi32), shard_cols(v_lo32), shard_cols(mask)], axis=-1
    )  # [NCORES, 128, NBLK, 3]
    vb = np.ascontiguousarray(vb).astype(mm_np)

    in_maps = []
    for c in range(NCORES):
        in_maps.append(
            {
                "xt": xT,
                "wqt": wqT,
                "keyst": np.ascontiguousarray(keysT[:, c * KC:(c + 1) * KC]),
                "vb": np.ascontiguousarray(vb[c].reshape(128, NBLK * 3)),
            }
        )
    return in_maps


def kernel(x, curiosity_score, Wq, keys, values):
    global LAST_RESULTS
    if TRACE:
        _install_ntff_hook()
    from concourse.bass_utils import run_bass_kernel_spmd

    key = (MM_DT, PROJ_DT)
    if key not in _cache:
        _cache[key] = _build(MM_DT, PROJ_DT)
    nc = _cache[key]

    in_maps = _prep_inputs(x, Wq, keys, values, MM_DT, PROJ_DT)

    res = run_bass_kernel_spmd(
        nc, in_maps, core_ids=list(range(NCORES)), trace=TRACE
    )
    LAST_RESULTS = res

    nd = np.stack(
        [np.asarray(res.results[c]["nd_out"], dtype=np.float64) for c in range(NCORES)]
    ).reshape(NCORES, 3, H, BT)
    num = (nd[:, 0] + nd[:, 1]).sum(axis=0)  # [H, BT]
    den = nd[:, 2].sum(axis=0)               # [H, BT]
    out = (num / den).mean(axis=0) + np.asarray(
        curiosity_score, dtype=np.float64
    ).reshape(BT)
    return out.astype(np.float32).reshape(B, T, 1)


# revision 9
# speedup vs baseline: 1.0532x; 1.0532x over previous
"""Trainium2 Bass kernel for nn_CrossAttention_44693429682227.

Math (reference):
    q = (x @ Wq.T) / E**0.25, reshaped (b, t, H, E)
    scores = q @ keys.T over a shared bank of N=50000 (key, scalar-value) pairs
    attn = softmax(scores, axis=-1)
    out = mean_h(attn @ values) + curiosity  -> (b, t, 1)

Because values are scalars, out_row = (sum_n exp(s_n) * v_n) / (sum_n exp(s_n)).
Scores are bounded (|s| <~ 20), so f32 exp never overflows and no max-
subtraction is needed; numerator and denominator partials are exact to merge
across key-bank shards.

Distribution: the key bank is sharded 8 ways (6528 keys/core incl. padding);
every core computes the projection for all 4096 (b,t,h) query rows (replicated,
cheap) and full partial num/den sums over its shard. Host merges partials.

Per-core program (all static/unrolled, Tile-scheduled):
  - PE: qT projection (8 heads x 8 k-chunks), then per (head, key-block):
        scoresT[block] = keysT_block.T @ qT_head   (out: [128 keys, 512 qrows])
        ndacc += vb_block.T @ exp(scoresT)         (vb = [v_hi, v_lo, mask])
  - ACT: exp over groups of GB=3 PSUM banks per instruction (amortizes the
    ~352-cycle ACTIVATE overhead)
  - output: [3, H, 512] num_hi/num_lo/den partials -> DRAM, merged on host

kernel.py is self-contained: shapes/sharding hardcoded, no sibling imports.
"""

import os
import sys
from contextlib import ExitStack

import numpy as np

if "/opt/trn_rl_repo" not in sys.path:
    sys.path.insert(0, "/opt/trn_rl_repo")

import ml_dtypes

# Problem shapes (hardcoded per contract)
B, T = 4, 128
BT = B * T            # 512 query (b,t) rows
HIN = 1024
H, E = 8, 128
N = 50000
NCORES = 8

# Sharding / tiling
GB = 3                # key-blocks (128 keys each) per ACTIVATE group
NGRP = 17             # groups per core
NBLK = GB * NGRP      # 51 key-blocks per core
KC = NBLK * 128       # 6528 keys per core
NPAD = KC * NCORES    # 52224 padded bank size
KCH = HIN // 128      # 8 contraction chunks for the projection

# dtype knobs: "bf16" | "f32r" | "f32"
MM_DT = os.environ.get("KMM_DT", "bf16")      # scores + num/den matmuls
PROJ_DT = os.environ.get("KPROJ_DT", "f32")   # q projection matmul
TRACE = bool(int(os.environ.get("KTRACE", "0")))

LAST_RESULTS = None   # BassKernelResults of the most recent run (for test.py)

_cache = {}


def _install_ntff_hook():
    """Register the axon NTFF profile hook that this image's antenv lacks.

    bass_utils reads it via ``antenv.axon_hooks.get_axon_ntff_profile_hook``;
    we synthesize that module around trn_agent_boot's ctypes implementation.
    Also soften ``upload_artifacts`` (no bucket access needed for local runs).
    """
    import types

    if "antenv.axon_hooks" in sys.modules:
        return
    try:
        from trn_agent_boot.trn_boot import _ntff_profile_via_ctypes

        hook = _ntff_profile_via_ctypes("/opt/axon/libaxon_pjrt.so")
    except Exception:
        hook = None
    mod = types.ModuleType("antenv.axon_hooks")
    mod.get_axon_ntff_profile_hook = lambda: hook
    sys.modules["antenv.axon_hooks"] = mod

    from concourse import bass_utils as bu

    orig_upload = bu.upload_artifacts

    def safe_upload(tmpdir):
        try:
            return orig_upload(tmpdir)
        except Exception as e:
            return f"upload-skipped ({type(e).__name__})"

    bu.upload_artifacts = safe_upload


def _np_dt(tag):
    return ml_dtypes.bfloat16 if tag == "bf16" else np.float32


def _build(mm, proj):
    import concourse.bass as bass
    import concourse.tile as tile
    from concourse import bacc, mybir

    f32 = mybir.dt.float32
    f32r = mybir.dt.float32r
    bf16 = mybir.dt.bfloat16

    def decl(tag):
        return {"bf16": bf16, "f32r": f32r, "f32": f32}[tag]

    def cast(ap, tag):
        return ap

    mm_decl = decl(mm)
    proj_decl = decl(proj)

    nc = bacc.Bacc(trn_type="TRN2", target_bir_lowering=False, debug=False)

    xt_d = nc.dram_tensor("xt", [HIN, BT], proj_decl, kind="ExternalInput")
    wqt_d = nc.dram_tensor("wqt", [HIN, H * E], proj_decl, kind="ExternalInput")
    keyst_d = nc.dram_tensor("keyst", [E, KC], mm_decl, kind="ExternalInput")
    vb_d = nc.dram_tensor("vb", [128, NBLK * 3], mm_decl, kind="ExternalInput")
    nd_d = nc.dram_tensor("nd_out", [3, H * BT], f32, kind="ExternalOutput")

    with tile.TileContext(nc) as tc, ExitStack() as ctx:
        singles = ctx.enter_context(tc.tile_pool(name="singles", bufs=1))
        epool = ctx.enter_context(tc.tile_pool(name="epool", bufs=3))
        ps_s = ctx.enter_context(tc.tile_pool(name="ps_s", bufs=2, space="PSUM"))
        ps_sm = ctx.enter_context(tc.tile_pool(name="ps_sm", bufs=2, space="PSUM"))

        # ---- persistent SBUF loads ----
        xt_sb = singles.tile([128, KCH, BT], proj_decl)
        for k in range(KCH):
            nc.sync.dma_start(out=xt_sb[:, k, :], in_=xt_d.ap()[128 * k:128 * (k + 1), :])

        wq_sb = singles.tile([128, KCH, H, E], proj_decl)
        wq_view = wqt_d.ap().rearrange("(k p) o -> p k o", p=128)
        for h in range(H):
            nc.sync.dma_start(out=wq_sb[:, :, h, :], in_=wq_view[:, :, E * h:E * (h + 1)])

        keyst_sb = singles.tile([128, KC], mm_decl)
        kchunk = KC // 8
        for i in range(8):
            nc.sync.dma_start(
                out=keyst_sb[:, kchunk * i:kchunk * (i + 1)],
                in_=keyst_d.ap()[:, kchunk * i:kchunk * (i + 1)],
            )

        vb_sb = singles.tile([128, NBLK, 3], mm_decl)
        nc.sync.dma_start(out=vb_sb, in_=vb_d.ap().rearrange("p (b c) -> p b c", c=3))

        qt_sb = singles.tile([128, H, BT], mm_decl)
        out_sb = singles.tile([3, H, BT], f32)

        # ---- per head: projection, then scores -> exp -> num/den ----
        # Projection is interleaved with the main loop so ACT (the floor
        # engine) starts consuming exp work within a few us of kernel start.
        Exp = mybir.ActivationFunctionType.Exp
        for h in range(H):
            q_ps = ps_s.tile([128, BT], f32, tag="s", name=f"q_ps{h}")
            for k in range(KCH):
                nc.tensor.matmul(
                    q_ps,
                    lhsT=cast(wq_sb[:, k, h, :], proj),
                    rhs=cast(xt_sb[:, k, :], proj),
                    start=(k == 0),
                    stop=(k == KCH - 1),
                )
            nc.vector.tensor_copy(qt_sb[:, h, :], q_ps)

            nd_ps = ps_sm.tile([3, BT], f32, tag="sm", name=f"nd_ps{h}")
            for g in range(NGRP):
                s_ps = ps_s.tile([128, GB, BT], f32, tag="s", name=f"s_ps_{h}_{g}")
                for j in range(GB):
                    b = g * GB + j
                    nc.tensor.matmul(
                        s_ps[:, j, :],
                        lhsT=cast(keyst_sb[:, 128 * b:128 * (b + 1)], mm),
                        rhs=cast(qt_sb[:, h, :], mm),
                        start=True,
                        stop=True,
                    )
                eT = epool.tile([128, GB, BT], mm_decl, tag="e", name=f"eT_{h}_{g}")
                nc.scalar.activation(eT, s_ps, Exp)
                for j in range(GB):
                    b = g * GB + j
                    nc.tensor.matmul(
                        nd_ps,
                        lhsT=cast(vb_sb[:, b, :], mm),
                        rhs=cast(eT[:, j, :], mm),
                        start=(b == 0),
                        stop=(b == NBLK - 1),
                    )
            nc.vector.tensor_copy(out_sb[:, h, :], nd_ps)

        nc.sync.dma_start(out=nd_d.ap(), in_=out_sb.rearrange("p h b -> p (h b)"))

    nc.compile()
    return nc


def _prep_inputs(x, Wq, keys, values, mm, proj):
    mm_np = _np_dt(mm)
    proj_np = _np_dt(proj)

    xT = np.ascontiguousarray(
        np.asarray(x, dtype=np.float32).reshape(BT, HIN).T
    ).astype(proj_np)
    # fold the 1/E**0.25 query scale into Wq
    wqT = np.ascontiguousarray(
        (np.asarray(Wq, dtype=np.float32) * np.float32(E ** -0.25)).T
    ).astype(proj_np)

    keys_pad = np.zeros((NPAD, E), dtype=np.float32)
    keys_pad[:N] = np.asarray(keys, dtype=np.float32)
    keysT = np.ascontiguousarray(keys_pad.T).astype(mm_np)  # [E, NPAD]

    v = np.asarray(values, dtype=np.float32)
    v_pad = np.zeros(NPAD, dtype=np.float32)
    v_pad[:N] = v
    mask = np.zeros(NPAD, dtype=np.float32)
    mask[:N] = 1.0

    if mm == "bf16":
        v_hi32 = v_pad.astype(ml_dtypes.bfloat16).astype(np.float32)
        v_lo32 = v_pad - v_hi32
    else:
        v_hi32 = v_pad
        v_lo32 = np.zeros_like(v_pad)

    # vb[core][p, blk, 3] with p = key index within 128-block
    def shard_cols(a):  # [NPAD] -> [NCORES, 128, NBLK]
        return a.reshape(NCORES, NBLK, 128).transpose(0, 2, 1)

    vb = np.stack(
        [shard_cols(v_hi32), shard_cols(v_lo32), shard_cols(mask)], axis=-1
    )  # [NCORES, 128, NBLK, 3]
    vb = np.ascontiguousarray(vb).astype(mm_np)

    in_maps = []
    for c in range(NCORES):
        in_maps.append(
            {
                "xt": xT,
                "wqt": wqT,
                "keyst": np.ascontiguousarray(keysT[:, c * KC:(c + 1) * KC]),
                "vb": np.ascontiguousarray(vb[c].reshape(128, NBLK * 3)),
            }
        )
    return in_maps


def kernel(x, curiosity_score, Wq, keys, values):
    global LAST_RESULTS
    if TRACE:
        _install_ntff_hook()
    from concourse.bass_utils import run_bass_kernel_spmd

    key = (MM_DT, PROJ_DT)
    if key not in _cache:
        _cache[key] = _build(MM_DT, PROJ_DT)
    nc = _cache[key]

    in_maps = _prep_inputs(x, Wq, keys, values, MM_DT, PROJ_DT)

    res = run_bass_kernel_spmd(
        nc, in_maps, core_ids=list(range(NCORES)), trace=TRACE
    )
    LAST_RESULTS = res

    nd = np.stack(
        [np.asarray(res.results[c]["nd_out"], dtype=np.float64) for c in range(NCORES)]
    ).reshape(NCORES, 3, H, BT)
    num = (nd[:, 0] + nd[:, 1]).sum(axis=0)  # [H, BT]
    den = nd[:, 2].sum(axis=0)               # [H, BT]
    out = (num / den).mean(axis=0) + np.asarray(
        curiosity_score, dtype=np.float64
    ).reshape(BT)
    return out.astype(np.float32).reshape(B, T, 1)


# revision 14
# speedup vs baseline: 1.1428x; 1.0851x over previous
"""Trainium2 Bass kernel for nn_CrossAttention_44693429682227.

Math (reference):
    q = (x @ Wq.T) / E**0.25, reshaped (b, t, H, E)
    scores = q @ keys.T over a shared bank of N=50000 (key, scalar-value) pairs
    attn = softmax(scores, axis=-1)
    out = mean_h(attn @ values) + curiosity  -> (b, t, 1)

Because values are scalars, out_row = (sum_n exp(s_n) * v_n) / (sum_n exp(s_n)).
Scores are bounded (|s| <~ 20), so f32 exp never overflows and no max-
subtraction is needed; numerator and denominator partials are exact to merge
across key-bank shards.

Distribution: the key bank is sharded 8 ways (6528 keys/core incl. padding);
every core computes the projection for all 4096 (b,t,h) query rows (replicated,
cheap) and full partial num/den sums over its shard. Host merges partials.

Per-core program (all static/unrolled, Tile-scheduled):
  - PE: qT projection (8 heads x 8 k-chunks), then per (head, key-block):
        scoresT[block] = keysT_block.T @ qT_head   (out: [128 keys, 512 qrows])
        ndacc += vb_block.T @ exp(scoresT)         (vb = [v_hi, v_lo, mask])
  - ACT: exp over groups of GB=3 PSUM banks per instruction (amortizes the
    ~352-cycle ACTIVATE overhead)
  - output: [3, H, 512] num_hi/num_lo/den partials -> DRAM, merged on host

kernel.py is self-contained: shapes/sharding hardcoded, no sibling imports.
"""

import os
import sys
from contextlib import ExitStack

import numpy as np

if "/opt/trn_rl_repo" not in sys.path:
    sys.path.insert(0, "/opt/trn_rl_repo")

import ml_dtypes

# Problem shapes (hardcoded per contract)
B, T = 4, 128
BT = B * T            # 512 query (b,t) rows
HIN = 1024
H, E = 8, 128
N = 50000
NCORES = 8

# Sharding / tiling
GB = 3                # key-blocks (128 keys each) per ACTIVATE group
NGRP = 17             # groups per core
NBLK = GB * NGRP      # 51 key-blocks per core
KC = NBLK * 128       # 6528 keys per core
NPAD = KC * NCORES    # 52224 padded bank size
KCH = HIN // 128      # 8 contraction chunks for the projection

# dtype knobs: "bf16" | "f32r" | "f32"
MM_DT = os.environ.get("KMM_DT", "bf16")      # scores + num/den matmuls
PROJ_DT = os.environ.get("KPROJ_DT", "f32")   # q projection matmul
TRACE = bool(int(os.environ.get("KTRACE", "0")))

LAST_RESULTS = None   # BassKernelResults of the most recent run (for test.py)

_cache = {}


def _install_ntff_hook():
    """Register the axon NTFF profile hook that this image's antenv lacks.

    bass_utils reads it via ``antenv.axon_hooks.get_axon_ntff_profile_hook``;
    we synthesize that module around trn_agent_boot's ctypes implementation.
    Also soften ``upload_artifacts`` (no bucket access needed for local runs).
    """
    import types

    if "antenv.axon_hooks" in sys.modules:
        return
    try:
        from trn_agent_boot.trn_boot import _ntff_profile_via_ctypes

        hook = _ntff_profile_via_ctypes("/opt/axon/libaxon_pjrt.so")
    except Exception:
        hook = None
    mod = types.ModuleType("antenv.axon_hooks")
    mod.get_axon_ntff_profile_hook = lambda: hook
    sys.modules["antenv.axon_hooks"] = mod

    from concourse import bass_utils as bu

    orig_upload = bu.upload_artifacts

    def safe_upload(tmpdir):
        try:
            return orig_upload(tmpdir)
        except Exception as e:
            return f"upload-skipped ({type(e).__name__})"

    bu.upload_artifacts = safe_upload


def _np_dt(tag):
    return ml_dtypes.bfloat16 if tag == "bf16" else np.float32


def _build(mm, proj):
    import concourse.bass as bass
    import concourse.tile as tile
    from concourse import bacc, mybir

    f32 = mybir.dt.float32
    f32r = mybir.dt.float32r
    bf16 = mybir.dt.bfloat16

    def decl(tag):
        return {"bf16": bf16, "f32r": f32r, "f32": f32}[tag]

    def cast(ap, tag):
        return ap

    mm_decl = decl(mm)
    proj_decl = decl(proj)

    nc = bacc.Bacc(trn_type="TRN2", target_bir_lowering=False, debug=False)

    # Host pre-arranges xt/wqt so every DMA is one contiguous run per
    # partition: xt[p, k, bt] = x[bt, 128k+p]; wqt[h, p, k, e] = Wq.T[128k+p, 128h+e]
    xt_d = nc.dram_tensor("xt", [128, KCH * BT], proj_decl, kind="ExternalInput")
    wqt_d = nc.dram_tensor("wqt", [H, 128, KCH * E], proj_decl, kind="ExternalInput")
    keyst_d = nc.dram_tensor("keyst", [E, KC], mm_decl, kind="ExternalInput")
    vb_d = nc.dram_tensor("vb", [128, NBLK * 3], mm_decl, kind="ExternalInput")
    nd_d = nc.dram_tensor("nd_out", [3, H * BT], f32, kind="ExternalOutput")

    with tile.TileContext(nc) as tc, ExitStack() as ctx:
        singles = ctx.enter_context(tc.tile_pool(name="singles", bufs=1))
        epool = ctx.enter_context(tc.tile_pool(name="epool", bufs=3))
        ps_s = ctx.enter_context(tc.tile_pool(name="ps_s", bufs=2, space="PSUM"))
        ps_q = ctx.enter_context(tc.tile_pool(name="ps_q", bufs=1, space="PSUM"))
        ps_sm = ctx.enter_context(tc.tile_pool(name="ps_sm", bufs=1, space="PSUM"))

        # ---- persistent SBUF loads ----
        # Order matters for the critical path: head 0's weights, then x,
        # then the rest; key bank streams behind them.
        wq_sb = singles.tile([128, H, KCH, E], proj_decl)
        for h in range(H):
            nc.sync.dma_start(
                out=wq_sb[:, h, :, :],
                in_=wqt_d.ap()[h].rearrange("p (k e) -> p k e", e=E),
            )
            if h == 0:
                xt_sb = singles.tile([128, KCH, BT], proj_decl)
                nc.sync.dma_start(
                    out=xt_sb, in_=xt_d.ap().rearrange("p (k b) -> p k b", b=BT)
                )

        keyst_sb = singles.tile([128, KC], mm_decl)
        kchunk = KC // 8
        for i in range(8):
            nc.sync.dma_start(
                out=keyst_sb[:, kchunk * i:kchunk * (i + 1)],
                in_=keyst_d.ap()[:, kchunk * i:kchunk * (i + 1)],
            )

        vb_sb = singles.tile([128, NBLK, 3], mm_decl)
        nc.sync.dma_start(out=vb_sb, in_=vb_d.ap().rearrange("p (b c) -> p b c", c=3))

        qt_sb = singles.tile([128, H, BT], mm_decl)
        out_sb = singles.tile([3, H, BT], f32)

        # ---- per head: scores -> exp -> num/den, with the NEXT head's
        # projection software-pipelined into group 0 so ACT never starves
        # at head boundaries.
        Exp = mybir.ActivationFunctionType.Exp

        def proj(h):
            q_ps = ps_q.tile([128, BT], f32, tag="q", name=f"q_ps{h}")
            for k in range(KCH):
                nc.tensor.matmul(
                    q_ps,
                    lhsT=cast(wq_sb[:, h, k, :], proj),
                    rhs=cast(xt_sb[:, k, :], proj),
                    start=(k == 0),
                    stop=(k == KCH - 1),
                )
            nc.vector.tensor_copy(qt_sb[:, h, :], q_ps)

        proj(0)
        for h in range(H):
            nd_ps = ps_sm.tile([3, BT], f32, tag="sm", name=f"nd_ps{h}")
            for g in range(NGRP):
                s_ps = ps_s.tile([128, GB, BT], f32, tag="s", name=f"s_ps_{h}_{g}")
                for j in range(GB):
                    b = g * GB + j
                    nc.tensor.matmul(
                        s_ps[:, j, :],
                        lhsT=cast(keyst_sb[:, 128 * b:128 * (b + 1)], mm),
                        rhs=cast(qt_sb[:, h, :], mm),
                        start=True,
                        stop=True,
                    )
                eT = epool.tile([128, GB, BT], mm_decl, tag="e", name=f"eT_{h}_{g}")
                nc.scalar.activation(eT, s_ps, Exp)
                for j in range(GB):
                    b = g * GB + j
                    nc.tensor.matmul(
                        nd_ps,
                        lhsT=cast(vb_sb[:, b, :], mm),
                        rhs=cast(eT[:, j, :], mm),
                        start=(b == 0),
                        stop=(b == NBLK - 1),
                    )
                if g == 0 and h + 1 < H:
                    proj(h + 1)
            nc.vector.tensor_copy(out_sb[:, h, :], nd_ps)

        nc.sync.dma_start(out=nd_d.ap(), in_=out_sb.rearrange("p h b -> p (h b)"))

    nc.compile()
    return nc


def _prep_inputs(x, Wq, keys, values, mm, proj):
    mm_np = _np_dt(mm)
    proj_np = _np_dt(proj)

    # xt[p, k, bt] = x[bt, 128k+p]  (one contiguous run per partition)
    xT = np.ascontiguousarray(
        np.asarray(x, dtype=np.float32).reshape(BT, KCH, 128).transpose(2, 1, 0)
    ).reshape(128, KCH * BT).astype(proj_np)
    # wqt[h, p, k, e] = Wq.T[128k+p, 128h+e], with 1/E**0.25 folded in
    wq_s = np.asarray(Wq, dtype=np.float32) * np.float32(E ** -0.25)  # [oc, hin]
    wqT = np.ascontiguousarray(
        wq_s.reshape(H, E, KCH, 128).transpose(0, 3, 2, 1)  # [h, p, k, e]
    ).reshape(H, 128, KCH * E).astype(proj_np)

    keys_pad = np.zeros((NPAD, E), dtype=np.float32)
    keys_pad[:N] = np.asarray(keys, dtype=np.float32)
    keysT = np.ascontiguousarray(keys_pad.T).astype(mm_np)  # [E, NPAD]

    v = np.asarray(values, dtype=np.float32)
    v_pad = np.zeros(NPAD, dtype=np.float32)
    v_pad[:N] = v
    mask = np.zeros(NPAD, dtype=np.float32)
    mask[:N] = 1.0

    if mm == "bf16":
        v_hi32 = v_pad.astype(ml_dtypes.bfloat16).astype(np.float32)
        v_lo32 = v_pad - v_hi32
    else:
        v_hi32 = v_pad
        v_lo32 = np.zeros_like(v_pad)

    # vb[core][p, blk, 3] with p = key index within 128-block
    def shard_cols(a):  # [NPAD] -> [NCORES, 128, NBLK]
        return a.reshape(NCORES, NBLK, 128).transpose(0, 2, 1)

    vb = np.stack(
        [shard_cols(v_hi32), shard_cols(v_lo32), shard_cols(mask)], axis=-1
    )  # [NCORES, 128, NBLK, 3]
    vb = np.ascontiguousarray(vb).astype(mm_np)

    in_maps = []
    for c in range(NCORES):
        in_maps.append(
            {
                "xt": xT,
                "wqt": wqT,
                "keyst": np.ascontiguousarray(keysT[:, c * KC:(c + 1) * KC]),
                "vb": np.ascontiguousarray(vb[c].reshape(128, NBLK * 3)),
            }
        )
    return in_maps


def kernel(x, curiosity_score, Wq, keys, values):
    global LAST_RESULTS
    if TRACE:
        _install_ntff_hook()
    from concourse.bass_utils import run_bass_kernel_spmd

    key = (MM_DT, PROJ_DT)
    if key not in _cache:
        _cache[key] = _build(MM_DT, PROJ_DT)
    nc = _cache[key]

    in_maps = _prep_inputs(x, Wq, keys, values, MM_DT, PROJ_DT)

    res = run_bass_kernel_spmd(
        nc, in_maps, core_ids=list(range(NCORES)), trace=TRACE
    )
    LAST_RESULTS = res

    nd = np.stack(
        [np.asarray(res.results[c]["nd_out"], dtype=np.float64) for c in range(NCORES)]
    ).reshape(NCORES, 3, H, BT)
    num = (nd[:, 0] + nd[:, 1]).sum(axis=0)  # [H, BT]
    den = nd[:, 2].sum(axis=0)               # [H, BT]
    out = (num / den).mean(axis=0) + np.asarray(
        curiosity_score, dtype=np.float64
    ).reshape(BT)
    return out.astype(np.float32).reshape(B, T, 1)
